# revision 72
# baseline (speedup 1.0000x reference)
"""Multi-head attention (softmax over query axis) on 8 Trainium2 cores.

Problem: nn_MultiHeadAttention_3899830305178
  B=2, S=2048, D_MODEL=1024, HEADS=16, D_K=64, fp32 IO.
  reference:
    q = (query @ Wq + bq), k = ..., v = ...        [b, s, h, dk]
    scores = einsum('bihd,bjhd->bijh', q, k) / 8
    attn = softmax(scores, axis=1)                 # over QUERY axis i (quirk)
    x = einsum('bijh,bjhd->bihd', attn, v)         [b, s, h*dk]
    out = x @ Wo + bo
Sharding: data-parallel over batch (2) x tensor-parallel over heads (4 groups
of 4 heads) = 8 cores. Each core computes a partial output
O_part = x_local @ Wo[rows of its heads]; the host sums the 4 partials per
batch (row-parallel unshard) -- bo is added on-device by the g==0 core.

Per-core kernel math (host passes query/key/value pre-transposed, bf16):
  qT[d', i] = Wq_s.T @ queryT  (+bq, bf16 out)   d' = 4 local heads x 64 = 256
  kT[d', j] = Wk_s.T @ keyT    (+bk, bf16 out)
  vT[d', j] = Wv_s.T @ valueT  (+bv, bf16), then bf16 DMA-transpose -> v[j, d']
  per head h:  sT[j, i] = kT_h.T @ qT_h / 8  (softmax over i == free axis)
               eT = exp(sT) in bf16, rowsum over i fused via ACT accum_out
               vsc[j, :] = v_h[j, :] / rowsum[j]  (bf16)
               x[hd, i] += vsc.T @ eT             (contracts over j strips)
  Heads are processed in pairs; both heads' x accumulate into one PSUM tile
  [128, S] (even head -> partitions 0-63, odd head -> 64-127) so the PSUM
  copy lands directly in the head-major xT layout the output projection
  needs (bf16).
  O = xT0.T @ Wo[0] (+bo) staged as bf16 partials during pair 1's attention,
  then xT1.T @ Wo[1] + partial (via an identity-weight matmul) at the tail.

Program order is hand-interleaved (projection chunks, early ih=0 exps, the
pair-0 partial output projection inside pair 1's loop) because the Tile
framework assigns PSUM pool slots in emission order. A short warm-up matmul
burst at t=0 brings the PE out of its low p-state before the first
projection chunk lands.

All matmul operands are bf16 (fp32 PSUM accumulation); softmax statistics
are fp32. End-to-end relative error vs the fp32 reference ~6e-3.
"""

import numpy as np

import concourse.bass as bass
import concourse.mybir as mybir
import concourse.tile as tile
from concourse.bass_utils import run_bass_kernel_spmd

# problem shape (hardcoded per contract)
B, S, DM, H, DK = 2, 2048, 1024, 16, 64
N_CORES = 8
GROUPS = 4              # head groups (tensor-parallel)
HL = H // GROUPS        # 4 local heads per core
DL = HL * DK            # 256 local concat width
P = 128
SJ = S // P             # 16 strips of 128 along j (keys) and i (out rows)
MT = DM // P            # 8 contraction tiles for projections
DPT = DL // P           # 2 partition tiles of the local concat dim
SCALE = 1.0 / 8.0       # 1/sqrt(DK)

f32 = mybir.dt.float32
bf16 = mybir.dt.bfloat16
AF = mybir.ActivationFunctionType

_PROGRAM = None


def _split_excess_waits(nc, max_waits=1):
    """walrus in this container rejects >1 semaphore wait per instruction
    (e.g. the Tile kernel-tail Drain); move extras onto same-engine NOPs."""
    n_split = 0
    for f in nc.m.functions:
        for blk in f.blocks:
            new_insts = []
            for inst in blk.instructions:
                si = getattr(inst, "sync_info", None)
                if si is not None and si.on_wait and len(si.on_wait) > max_waits:
                    waits = list(si.on_wait)
                    extra, keep = waits[:-max_waits], waits[-max_waits:]
                    for i in range(0, len(extra), max_waits):
                        chunk = extra[i:i + max_waits]
                        nop = mybir.InstNoOp(
                            name=f"{inst.name}-ws{n_split}-{i}",
                            engine=inst.engine,
                            sync_info=mybir.SyncInfo(on_wait=chunk, on_update=[]),
                            bass_nofuse=True,
                        )
                        new_insts.append(nop)
                    si.on_wait = keep
                    n_split += 1
                new_insts.append(inst)
            blk.instructions[:] = new_insts
    return n_split


def build_program(split_waits=True):
    nc = bass.Bass("TRN2", target_bir_lowering=False, debug=False)

    qT_in = nc.dram_tensor("qT_in", [DM, S], bf16, kind="ExternalInput")
    kT_in = nc.dram_tensor("kT_in", [DM, S], bf16, kind="ExternalInput")
    vT_in = nc.dram_tensor("vT_in", [DM, S], bf16, kind="ExternalInput")
    wq_d = nc.dram_tensor("wq", [DM, DL], bf16, kind="ExternalInput")
    wk_d = nc.dram_tensor("wk", [DM, DL], bf16, kind="ExternalInput")
    wv_d = nc.dram_tensor("wv", [DM, DL], bf16, kind="ExternalInput")
    wo_d = nc.dram_tensor("wo", [DL, DM], bf16, kind="ExternalInput")
    bq_d = nc.dram_tensor("bq", [DL, 1], f32, kind="ExternalInput")
    bk_d = nc.dram_tensor("bk", [DL, 1], f32, kind="ExternalInput")
    bv_d = nc.dram_tensor("bv", [DL, 1], f32, kind="ExternalInput")
    bo_d = nc.dram_tensor("bo_bc", [P, DM], f32, kind="ExternalInput")
    O0_d = nc.dram_tensor("O0", [S, DM], bf16, kind="ExternalOutput")
    O1_d = nc.dram_tensor("O1", [S, DM], bf16, kind="ExternalOutput")

    with tile.TileContext(nc) as tc:
        with (
            tc.tile_pool(name="const", bufs=1) as const,
            tc.tile_pool(name="persist", bufs=1) as sb,
            tc.tile_pool(name="stat", bufs=6) as stat,
            tc.tile_pool(name="outp", bufs=4) as outp,
            tc.tile_pool(name="inp", bufs=4) as inp,
            tc.tile_pool(name="vtp", bufs=1) as vtp,
            tc.tile_pool(name="attp", bufs=46) as attp,
            tc.tile_pool(name="pps", bufs=2, space="PSUM") as pps,
            tc.tile_pool(name="ppx", bufs=1, space="PSUM") as ppx,
        ):
            # ---------------- PE warm-up ----------------
            # The tensor engine p-state ramps with sustained use; a burst of
            # dependency-free matmuls at t=0 brings it to full clock while
            # the first input DMAs are still in flight.
            warm = const.tile([P, P], bf16, name="warm", tag="warm")
            nc.vector.memset(warm[:], 0.0)
            wps = pps.tile([P, P], f32, name="warmps", tag="ps")
            for w in range(20):
                nc.tensor.matmul(wps[:], warm[:], warm[:],
                                 start=True, stop=True,
                                 skip_group_check=True)

            # ---------------- constants ----------------
            # Weight DMAs land as DRAM [(t p), c] -> SBUF [p, (t c)] so the
            # m-th 128-row block sits at free offset m*cols; loaded in two
            # 4-m-block halves interleaved with the first input chunks.
            def load_w_half(t, dram, cols, half):
                src = dram.ap().rearrange("(t p) c -> p t c", p=P)
                nc.sync.dma_start(
                    t[:, half * 4 * cols:(half + 1) * 4 * cols]
                    .rearrange("p (t c) -> p t c", t=4),
                    src[:, half * 4:(half + 1) * 4, :])

            wq_sb = const.tile([P, MT * DL], bf16, name="wq", tag="wq")
            wk_sb = const.tile([P, MT * DL], bf16, name="wk", tag="wk")
            wv_sb = const.tile([P, MT * DL], bf16, name="wv", tag="wv")
            bq_sb = const.tile([P, DPT], f32, name="bq", tag="bq")
            bk_sb = const.tile([P, DPT], f32, name="bk", tag="bk")
            bv_sb = const.tile([P, DPT], f32, name="bv", tag="bv")

            def load_biases():
                for b_sb, b_d in ((bq_sb, bq_d), (bk_sb, bk_d),
                                  (bv_sb, bv_d)):
                    nc.sync.dma_start(
                        b_sb[:].rearrange("p (t c) -> p t c", t=DPT),
                        b_d.ap().rearrange("(t p) c -> p t c", p=P))

            def w_slice(w, m, dp):
                return w[:, m * DL + dp * P:m * DL + (dp + 1) * P]

            # ---------------- persistent activations ----------------
            qT_sb = [[sb.tile([P, 1024], bf16, name=f"qT{dp}_{ih}",
                              tag=f"qT{dp}_{ih}") for ih in range(2)]
                     for dp in range(DPT)]
            kT_sb = [[sb.tile([P, 512], bf16, name=f"kT{dp}_{jg}",
                              tag=f"kT{dp}_{jg}") for jg in range(4)]
                     for dp in range(DPT)]
            # v packed per j-group of 4: v4_sb[jg][p, jj*DL + d'] holds
            # v[jg*512 + jj*128 + p, d']
            v4_sb = [sb.tile([P, 4 * DL], bf16, name=f"v{jg}", tag=f"v{jg}")
                     for jg in range(4)]
            # xT packed head-major: xT_sb[hp] partitions = [h=2hp d 0..63,
            # h=2hp+1 d 0..63], matching the wo row layout.
            xT_sb = [sb.tile([P, S], bf16, name=f"xT{hp}", tag=f"xT{hp}")
                     for hp in range(DPT)]
            # vT staging is per (i4, dp): a [128, 512] tile lives only from
            # the bias-add to its DMA-transpose.
            vt_cur = {}

            # ---------------- building blocks ----------------
            def load_in_chunk(win, nm, i4):
                # two DMAs (4 m-blocks each): finer DMA->matmul pipelining
                t = inp.tile([P, MT * 512], bf16, name=f"{nm}in{i4}",
                             tag="pin")
                src = win.ap().rearrange("(t p) c -> p t c", p=P)
                for half in range(2):
                    nc.sync.dma_start(
                        t[:, half * 4 * 512:(half + 1) * 4 * 512]
                        .rearrange("p (t c) -> p t c", t=4),
                        src[:, half * 4:(half + 1) * 4,
                            i4 * 512:(i4 + 1) * 512])
                return t

            def proj_chunk(nm, ch, i4, w_sb, b_sb):
                # q/k projection psums share the "ps" tag (bufs=2) so chunk
                # n+1's matmuls overlap chunk n's bias-adds; v psums use the
                # "px" slot so they don't sit between the early exps and the
                # pair-0 ih=1 scores in the "ps" slot cycle.
                pool, tag = (ppx, "px") if nm == "v" else (pps, "ps")
                ps = pool.tile([P, 1024], f32, name=f"ps{nm}{i4}", tag=tag)
                for dp in range(DPT):
                    for m in range(MT):
                        nc.tensor.matmul(
                            ps[:, dp * 512:(dp + 1) * 512],
                            w_slice(w_sb, m, dp),
                            ch[:, m * 512:(m + 1) * 512],
                            start=(m == 0), stop=(m == MT - 1))
                for dp in range(DPT):
                    if nm == "q":
                        dst = qT_sb[dp][i4 // 2][:, (i4 % 2) * 512:
                                                 (i4 % 2) * 512 + 512]
                    elif nm == "k":
                        dst = kT_sb[dp][i4][:]
                    else:
                        t = vtp.tile([P, 512], bf16, name=f"vT{i4}_{dp}",
                                     tag="vt", bufs=4)
                        vt_cur[(i4, dp)] = t
                        dst = t[:]
                    nc.vector.tensor_scalar_add(
                        dst, ps[:, dp * 512:(dp + 1) * 512],
                        b_sb[:, dp:dp + 1])

            def scores_exp(h, j, ih):
                hp, hh = divmod(h, 2)
                base = hh * 64
                jg, jr = divmod(j, 4)
                a = attp.tile([P, 1024], bf16, name=f"att{h}_{j}_{ih}",
                              tag="att")
                ps = pps.tile([P, 1024], f32, name=f"pss{h}_{j}_{ih}",
                              tag="ps")
                for i5 in range(2):
                    io = i5 * 512
                    nc.tensor.matmul(
                        ps[:, io:io + 512],
                        kT_sb[hp][jg][base:base + 64, jr * P:(jr + 1) * P],
                        qT_sb[hp][ih][base:base + 64, io:io + 512],
                        start=True, stop=True)
                rsh = stat.tile([P, 1], f32, name=f"rsh{h}_{j}_{ih}",
                                tag="rsh", bufs=100)
                nc.scalar.activation(a[:], ps[:], AF.Exp, scale=SCALE,
                                     accum_out=rsh[:])
                return a, rsh

            def scores_exp_half(h, j, ih, i5, a):
                # 512-wide variant for the warm-up prefetch: lets the first
                # exps start after only q i4=0 and k i4=0 have projected.
                hp, hh = divmod(h, 2)
                base = hh * 64
                jg, jr = divmod(j, 4)
                io = i5 * 512
                ps = pps.tile([P, 512], f32, name=f"psh{h}_{j}_{ih}_{i5}",
                              tag="ps")
                nc.tensor.matmul(
                    ps[:],
                    kT_sb[hp][jg][base:base + 64, jr * P:(jr + 1) * P],
                    qT_sb[hp][ih][base:base + 64, io:io + 512],
                    start=True, stop=True)
                rsh = stat.tile([P, 1], f32, name=f"rshh{h}_{j}_{ih}_{i5}",
                                tag="rsh", bufs=100)
                nc.scalar.activation(a[:, io:io + 512], ps[:], AF.Exp,
                                     scale=SCALE, accum_out=rsh[:])
                return rsh

            def finish_strip(h, j, xps, ah, rs_halves):
                # xps: pair PSUM tile [128, S]; even head -> partitions 0:64,
                # odd head -> 64:128 (tile_position picked up automatically
                # from out.base_partition()).
                jg, jr = divmod(j, 4)
                po = (h % 2) * 64
                acc = rs_halves[0]
                for n, part in enumerate(rs_halves[1:]):
                    rs = stat.tile([P, 1], f32, name=f"rs{h}_{j}_{n}",
                                   tag="rs")
                    nc.vector.tensor_add(rs[:], acc[:], part[:])
                    acc = rs
                rs = acc
                rc = stat.tile([P, 1], f32, name=f"rc{h}_{j}", tag="rc")
                nc.vector.reciprocal(rc[:], rs[:])
                vsc = attp.tile([P, 64], bf16, name=f"vsc{h}_{j}", tag="vsc",
                                bufs=4)
                nc.vector.tensor_scalar_mul(
                    vsc[:],
                    v4_sb[jg][:, jr * DL + h * 64:jr * DL + (h + 1) * 64],
                    rc[:])
                for i5 in range(4):
                    io = (i5 % 2) * 512
                    nc.tensor.matmul(
                        xps[po:po + 64, i5 * 512:(i5 + 1) * 512], vsc[:],
                        ah[i5 // 2][:, io:io + 512],
                        start=(j == 0), stop=(j == SJ - 1),
                        skip_group_check=True)

            # ---------------- phase 1: q/k projections + ih=0 scores -------
            # Emission order is the PSUM slot order, so the early exps are
            # emitted between projection chunks as 512-wide halves, each as
            # soon as the qT/kT pieces it needs have projected.
            load_w_half(wq_sb, wq_d, DL, 0)
            load_w_half(wq_sb, wq_d, DL, 1)
            ch_q0 = load_in_chunk(qT_in, "q", 0)
            load_w_half(wk_sb, wk_d, DL, 0)
            load_w_half(wk_sb, wk_d, DL, 1)
            ch_k0 = load_in_chunk(kT_in, "k", 0)
            ch_q1 = load_in_chunk(qT_in, "q", 1)
            load_biases()

            saved = {}

            def emit_ih0_half(jg, i5):
                for j in range(jg * 4, jg * 4 + 4):
                    for h in (0, 1):
                        if (h, j) not in saved:
                            a = attp.tile([P, 1024], bf16,
                                          name=f"att{h}_{j}_0", tag="att")
                            saved[(h, j)] = (a, [])
                        a, parts = saved[(h, j)]
                        parts.append(scores_exp_half(h, j, 0, i5, a))

            def emit_ih0(jg):
                for j in range(jg * 4, jg * 4 + 4):
                    for h in (0, 1):
                        a, rsh = scores_exp(h, j, 0)
                        saved[(h, j)] = (a, [rsh])

            proj_chunk("q", ch_q0, 0, wq_sb, bq_sb)
            proj_chunk("k", ch_k0, 0, wk_sb, bk_sb)
            emit_ih0_half(0, 0)
            proj_chunk("q", ch_q1, 1, wq_sb, bq_sb)
            emit_ih0_half(0, 1)
            # remaining k chunks come before q i4=2,3: every ih=0 exp only
            # needs kT plus the first qT half, while the q tail is not needed
            # until the pair-0 main loop.
            # saved2: a few of pair 1's ih=0 exps pulled into phase-1 ACT
            # idle windows (they only need kT groups already projected).
            saved2 = {}

            def emit_pair1_ih0(h, j):
                saved2[(h, j)] = scores_exp(h, j, 0)

            ch = load_in_chunk(kT_in, "k", 1)
            proj_chunk("k", ch, 1, wk_sb, bk_sb)
            emit_ih0(1)
            ch = load_in_chunk(kT_in, "k", 2)
            proj_chunk("k", ch, 2, wk_sb, bk_sb)
            emit_ih0(2)
            ch = load_in_chunk(qT_in, "q", 2)
            proj_chunk("q", ch, 2, wq_sb, bq_sb)
            ch = load_in_chunk(kT_in, "k", 3)
            proj_chunk("k", ch, 3, wk_sb, bk_sb)
            emit_ih0(3)
            # first ih=1 half-exps (pair 0, j<4) fill the ACT window while
            # q i4=3 is still projecting
            saved1 = {}
            for j in range(4):
                for h in (0, 1):
                    a = attp.tile([P, 1024], bf16, name=f"att{h}_{j}_1",
                                  tag="att")
                    saved1[(h, j)] = (a, [scores_exp_half(h, j, 1, 0, a)])
            ch = load_in_chunk(qT_in, "q", 3)
            proj_chunk("q", ch, 3, wq_sb, bq_sb)
            for j in range(4):
                for h in (0, 1):
                    a, parts = saved1[(h, j)]
                    parts.append(scores_exp_half(h, j, 1, 1, a))

            # ---------------- phase 2: v projection + transposes ----------
            load_w_half(wv_sb, wv_d, DL, 0)
            load_w_half(wv_sb, wv_d, DL, 1)
            v_chunks = [load_in_chunk(vT_in, "v", i4) for i4 in range(3)]
            # wo/bo are first needed by the pair-0 partial output projection
            # (mid-kernel); loading them here keeps the front DMA bandwidth
            # for the projection inputs.
            wo_sb = const.tile([P, DPT * DM], bf16, name="wo", tag="wo")
            nc.sync.dma_start(
                wo_sb[:].rearrange("p (t c) -> p t c", t=DPT),
                wo_d.ap().rearrange("(t p) c -> p t c", p=P))
            bo_sb = const.tile([P, DM], f32, name="bo", tag="bo")
            nc.sync.dma_start(bo_sb[:], bo_d.ap())

            for i4 in range(4):
                if i4 == 1:
                    v_chunks.append(load_in_chunk(vT_in, "v", 3))
                proj_chunk("v", v_chunks[i4], i4, wv_sb, bv_sb)
                for dp in range(DPT):
                    out_view = v4_sb[i4][:].rearrange(
                        "p (j c) -> p j c", j=4)[:, :,
                                                 dp * P:(dp + 1) * P]
                    nc.sync.dma_start(
                        out_view, vt_cur.pop((i4, dp))[:], transpose=True)

            # ---------------- phase 3: attention (head pairs) -------------
            # finish_strip (softmax divisor + AV) runs one (h, j) step behind
            # scores/exp; pair 0's partial output projection (bf16, +bo) is
            # interleaved into pair 1's loop and streamed to DRAM there --
            # the tail only has pair 1's partial left to write. The host
            # sums the two bf16 partials in fp32.
            def o0_group(jt):
                ot = outp.tile([P, DM], bf16, name=f"o0_{jt}", tag="ot")
                ps = pps.tile([P, DM], f32, name=f"pso0_{jt}", tag="ps")
                for n5 in range(2):
                    no = n5 * 512
                    nc.tensor.matmul(
                        ps[:, no:no + 512], xT_sb[0][:, jt * P:(jt + 1) * P],
                        wo_sb[:, no:no + 512], start=True, stop=True)
                nc.vector.tensor_add(ot[:], ps[:], bo_sb[:])
                eng = (nc.sync, nc.scalar)[jt % 2]
                eng.dma_start(O0_d.ap()[jt * P:(jt + 1) * P, :], ot[:])

            for hp in range(DPT):
                h0, h1 = 2 * hp, 2 * hp + 1
                xps = ppx.tile([P, S], f32, name=f"xps{hp}", tag="px")
                pend = []
                step = 0
                for j in range(SJ):
                    for h in (h0, h1):
                        if hp == 0:
                            a0, r0parts = saved.pop((h, j))
                        elif (h, j) in saved2:
                            a0, rsh0 = saved2.pop((h, j))
                            r0parts = [rsh0]
                        else:
                            a0, rsh0 = scores_exp(h, j, 0)
                            r0parts = [rsh0]
                        if hp == 0 and (h, j) in saved1:
                            a1, r1parts = saved1.pop((h, j))
                        else:
                            a1, rsh1 = scores_exp(h, j, 1)
                            r1parts = [rsh1]
                        pend.append((h, j, a0, a1, r0parts + r1parts))
                        if len(pend) > 1:
                            ph, pj, b0, b1, rparts = pend.pop(0)
                            finish_strip(ph, pj, xps, [b0, b1], rparts)
                        if hp == 1 and step % 2 == 0 and step // 2 < SJ:
                            o0_group(step // 2)
                        step += 1
                for ph, pj, b0, b1, rparts in pend:
                    finish_strip(ph, pj, xps, [b0, b1], rparts)
                for c4 in range(4):
                    dst = xT_sb[hp][:, c4 * 512:(c4 + 1) * 512]
                    src = xps[:, c4 * 512:(c4 + 1) * 512]
                    if hp == 1 and c4 % 2 == 1:
                        # ACT is idle once the last exp is done
                        nc.scalar.activation(dst, src, AF.Copy)
                    else:
                        nc.vector.tensor_copy(dst, src)

            # ---------------- phase 4: output tail ------------------------
            # Pair 1's partial only: copies split DVE/ACT, bf16 DMAs rotated
            # across queues.
            for jt in range(SJ):
                ot = outp.tile([P, DM], bf16, name=f"ot{jt}", tag="ot")
                ps = pps.tile([P, DM], f32, name=f"pso1_{jt}", tag="ps")
                for n5 in range(2):
                    no = n5 * 512
                    nc.tensor.matmul(
                        ps[:, no:no + 512],
                        xT_sb[1][:, jt * P:(jt + 1) * P],
                        wo_sb[:, DM + no:DM + no + 512],
                        start=True, stop=True)
                if jt % 2 == 0:
                    nc.vector.tensor_copy(ot[:], ps[:])
                else:
                    nc.scalar.activation(ot[:], ps[:], AF.Copy)
                eng = (nc.sync, nc.gpsimd)[jt % 2]
                eng.dma_start(O1_d.ap()[jt * P:(jt + 1) * P, :], ot[:])

    if split_waits:
        _split_excess_waits(nc)
    return nc


def _get_program():
    global _PROGRAM
    if _PROGRAM is None:
        _PROGRAM = build_program()
    return _PROGRAM


def shard_inputs(inputs):
    """FULL inputs -> per-core in_maps (list of 8 dicts)."""
    import ml_dtypes

    def _bf16(x):
        return np.ascontiguousarray(np.asarray(x, np.float32)).astype(
            ml_dtypes.bfloat16)

    q = np.asarray(inputs["query"], dtype=np.float32)
    k = np.asarray(inputs["key"], dtype=np.float32)
    v = np.asarray(inputs["value"], dtype=np.float32)
    Wq = np.asarray(inputs["Wq"], dtype=np.float32)
    Wk = np.asarray(inputs["Wk"], dtype=np.float32)
    Wv = np.asarray(inputs["Wv"], dtype=np.float32)
    Wo = np.asarray(inputs["Wo"], dtype=np.float32)
    bq = np.asarray(inputs["bq"], dtype=np.float32)
    bk = np.asarray(inputs["bk"], dtype=np.float32)
    bv = np.asarray(inputs["bv"], dtype=np.float32)
    bo = np.asarray(inputs["bo"], dtype=np.float32)

    qT = [_bf16(q[b].T) for b in range(B)]
    kT = [_bf16(k[b].T) for b in range(B)]
    vT = [_bf16(v[b].T) for b in range(B)]

    in_maps = []
    for c in range(N_CORES):
        b, g = c // GROUPS, c % GROUPS
        sl = slice(g * DL, (g + 1) * DL)
        bo_bc = (np.ascontiguousarray(np.broadcast_to(bo, (P, DM)))
                 if g == 0 else np.zeros((P, DM), np.float32))
        in_maps.append({
            "qT_in": qT[b],
            "kT_in": kT[b],
            "vT_in": vT[b],
            "wq": _bf16(Wq[:, sl]),
            "wk": _bf16(Wk[:, sl]),
            "wv": _bf16(Wv[:, sl]),
            "wo": _bf16(Wo[sl, :]),
            "bq": np.ascontiguousarray(bq[sl].reshape(DL, 1)),
            "bk": np.ascontiguousarray(bk[sl].reshape(DL, 1)),
            "bv": np.ascontiguousarray(bv[sl].reshape(DL, 1)),
            "bo_bc": bo_bc,
        })
    return in_maps


def unshard_output(results):
    """results: list of 8 dicts with bf16 partials 'O0'/'O1' [S, DM] ->
    full [B, S, DM] fp32."""
    out = np.zeros((B, S, DM), np.float32)
    for c in range(N_CORES):
        b = c // GROUPS
        out[b] += np.asarray(results[c]["O0"], np.float32)
        out[b] += np.asarray(results[c]["O1"], np.float32)
    return out


def kernel(**inputs):
    nc = _get_program()
    in_maps = shard_inputs(inputs)
    res = run_bass_kernel_spmd(nc, in_maps, core_ids=list(range(N_CORES)))
    return unshard_output(res.results)


# revision 82
# speedup vs baseline: 2.3104x; 2.3104x over previous
"""Multi-head attention (softmax over query axis) on 8 Trainium2 cores.

Problem: nn_MultiHeadAttention_3899830305178
  B=2, S=2048, D_MODEL=1024, HEADS=16, D_K=64, fp32 IO.
  reference:
    q = (query @ Wq + bq), k = ..., v = ...        [b, s, h, dk]
    scores = einsum('bihd,bjhd->bijh', q, k) / 8
    attn = softmax(scores, axis=1)                 # over QUERY axis i (quirk)
    x = einsum('bijh,bjhd->bihd', attn, v)         [b, s, h*dk]
    out = x @ Wo + bo
Sharding: data-parallel over batch (2) x tensor-parallel over heads (4 groups
of 4 heads) = 8 cores. Each core computes a partial output
O_part = x_local @ Wo[rows of its heads]; the host sums the 4 partials per
batch (row-parallel unshard) -- bo is added on-device by the g==0 core.

Per-core kernel math (host passes query/key/value pre-transposed, bf16):
  qT[d', i] = Wq_s.T @ queryT  (+bq, bf16 out)   d' = 4 local heads x 64 = 256
  kT[d', j] = Wk_s.T @ keyT    (+bk, bf16 out)
  vT[d', j] = Wv_s.T @ valueT  (+bv, bf16), then bf16 DMA-transpose -> v[j, d']
  per head h:  sT[j, i] = kT_h.T @ qT_h / 8  (softmax over i == free axis)
               eT = exp(sT) in bf16, rowsum over i fused via ACT accum_out
               vsc[j, :] = v_h[j, :] / rowsum[j]  (bf16)
               x[hd, i] += vsc.T @ eT             (contracts over j strips)
  Heads are processed in pairs; both heads' x accumulate into one PSUM tile
  [128, S] (even head -> partitions 0-63, odd head -> 64-127) so the PSUM
  copy lands directly in the head-major xT layout the output projection
  needs (bf16).
  The output projection is split by pair into two bf16 DRAM partials:
  O0 = xT0.T @ Wo[0] + bo streams out during pair 1's attention, O1 =
  xT1.T @ Wo[1] at the tail; the host sums them in fp32 (so only 4 MB of
  the 8 MB output lands on the serial end-of-kernel DMA path).

Program order is hand-interleaved (projection chunks, early ih=0 exps, the
pair-0 partial output projection inside pair 1's loop) because the Tile
framework assigns PSUM pool slots in emission order. A short warm-up matmul
burst at t=0 brings the PE out of its low p-state before the first
projection chunk lands.

All matmul operands are bf16 (fp32 PSUM accumulation); softmax statistics
are fp32. End-to-end relative error vs the fp32 reference ~6e-3.
"""

import numpy as np

import concourse.bass as bass
import concourse.mybir as mybir
import concourse.tile as tile
from concourse.bass_utils import run_bass_kernel_spmd

# problem shape (hardcoded per contract)
B, S, DM, H, DK = 2, 2048, 1024, 16, 64
N_CORES = 8
GROUPS = 4              # head groups (tensor-parallel)
HL = H // GROUPS        # 4 local heads per core
DL = HL * DK            # 256 local concat width
P = 128
SJ = S // P             # 16 strips of 128 along j (keys) and i (out rows)
MT = DM // P            # 8 contraction tiles for projections
DPT = DL // P           # 2 partition tiles of the local concat dim
SCALE = 1.0 / 8.0       # 1/sqrt(DK)

f32 = mybir.dt.float32
bf16 = mybir.dt.bfloat16
AF = mybir.ActivationFunctionType

_PROGRAM = None


def _split_excess_waits(nc, max_waits=1):
    """walrus in this container rejects >1 semaphore wait per instruction
    (e.g. the Tile kernel-tail Drain); move extras onto same-engine NOPs."""
    n_split = 0
    for f in nc.m.functions:
        for blk in f.blocks:
            new_insts = []
            for inst in blk.instructions:
                si = getattr(inst, "sync_info", None)
                if si is not None and si.on_wait and len(si.on_wait) > max_waits:
                    waits = list(si.on_wait)
                    extra, keep = waits[:-max_waits], waits[-max_waits:]
                    for i in range(0, len(extra), max_waits):
                        chunk = extra[i:i + max_waits]
                        nop = mybir.InstNoOp(
                            name=f"{inst.name}-ws{n_split}-{i}",
                            engine=inst.engine,
                            sync_info=mybir.SyncInfo(on_wait=chunk, on_update=[]),
                            bass_nofuse=True,
                        )
                        new_insts.append(nop)
                    si.on_wait = keep
                    n_split += 1
                new_insts.append(inst)
            blk.instructions[:] = new_insts
    return n_split


def build_program(split_waits=True):
    nc = bass.Bass("TRN2", target_bir_lowering=False, debug=False)

    qT_in = nc.dram_tensor("qT_in", [DM, S], bf16, kind="ExternalInput")
    kT_in = nc.dram_tensor("kT_in", [DM, S], bf16, kind="ExternalInput")
    vT_in = nc.dram_tensor("vT_in", [DM, S], bf16, kind="ExternalInput")
    wq_d = nc.dram_tensor("wq", [DM, DL], bf16, kind="ExternalInput")
    wk_d = nc.dram_tensor("wk", [DM, DL], bf16, kind="ExternalInput")
    wv_d = nc.dram_tensor("wv", [DM, DL], bf16, kind="ExternalInput")
    wo_d = nc.dram_tensor("wo", [DL, DM], bf16, kind="ExternalInput")
    bq_d = nc.dram_tensor("bq", [DL, 1], f32, kind="ExternalInput")
    bk_d = nc.dram_tensor("bk", [DL, 1], f32, kind="ExternalInput")
    bv_d = nc.dram_tensor("bv", [DL, 1], f32, kind="ExternalInput")
    bo_d = nc.dram_tensor("bo_bc", [P, DM], f32, kind="ExternalInput")
    O0_d = nc.dram_tensor("O0", [S, DM], bf16, kind="ExternalOutput")
    O1_d = nc.dram_tensor("O1", [S, DM], bf16, kind="ExternalOutput")

    with tile.TileContext(nc) as tc:
        with (
            tc.tile_pool(name="const", bufs=1) as const,
            tc.tile_pool(name="persist", bufs=1) as sb,
            tc.tile_pool(name="stat", bufs=6) as stat,
            tc.tile_pool(name="outp", bufs=4) as outp,
            tc.tile_pool(name="inp", bufs=4) as inp,
            tc.tile_pool(name="vtp", bufs=1) as vtp,
            tc.tile_pool(name="attp", bufs=48) as attp,
            tc.tile_pool(name="pps", bufs=2, space="PSUM") as pps,
            tc.tile_pool(name="ppx", bufs=1, space="PSUM") as ppx,
        ):
            # ---------------- PE warm-up ----------------
            # The tensor engine p-state ramps with sustained use; a burst of
            # dependency-free matmuls at t=0 brings it to full clock while
            # the first input DMAs are still in flight.
            warm = const.tile([P, P], bf16, name="warm", tag="warm")
            nc.vector.memset(warm[:], 0.0)
            # dummy exp pulls the ACT exp-table load (~1.3-2.7us) off the
            # first real exp's critical path
            wexp = const.tile([P, 1], f32, name="wexp", tag="wexp")
            nc.scalar.activation(wexp[:], warm[:, 0:1], AF.Exp)
            wps = pps.tile([P, P], f32, name="warmps", tag="ps")
            for w in range(20):
                nc.tensor.matmul(wps[:], warm[:], warm[:],
                                 start=True, stop=True,
                                 skip_group_check=True)

            # ---------------- constants ----------------
            # Weight DMAs land as DRAM [(t p), c] -> SBUF [p, (t c)] so the
            # m-th 128-row block sits at free offset m*cols; loaded in two
            # 4-m-block halves interleaved with the first input chunks.
            def load_w_half(t, dram, cols, half):
                src = dram.ap().rearrange("(t p) c -> p t c", p=P)
                nc.sync.dma_start(
                    t[:, half * 4 * cols:(half + 1) * 4 * cols]
                    .rearrange("p (t c) -> p t c", t=4),
                    src[:, half * 4:(half + 1) * 4, :])

            wq_sb = const.tile([P, MT * DL], bf16, name="wq", tag="wq")
            wk_sb = const.tile([P, MT * DL], bf16, name="wk", tag="wk")
            wv_sb = const.tile([P, MT * DL], bf16, name="wv", tag="wv")
            bq_sb = const.tile([P, DPT], f32, name="bq", tag="bq")
            bk_sb = const.tile([P, DPT], f32, name="bk", tag="bk")
            bv_sb = const.tile([P, DPT], f32, name="bv", tag="bv")

            def load_biases():
                for b_sb, b_d in ((bq_sb, bq_d), (bk_sb, bk_d),
                                  (bv_sb, bv_d)):
                    nc.sync.dma_start(
                        b_sb[:].rearrange("p (t c) -> p t c", t=DPT),
                        b_d.ap().rearrange("(t p) c -> p t c", p=P))

            def w_slice(w, m, dp):
                return w[:, m * DL + dp * P:m * DL + (dp + 1) * P]

            # ---------------- persistent activations ----------------
            qT_sb = [[sb.tile([P, 1024], bf16, name=f"qT{dp}_{ih}",
                              tag=f"qT{dp}_{ih}") for ih in range(2)]
                     for dp in range(DPT)]
            kT_sb = [[sb.tile([P, 512], bf16, name=f"kT{dp}_{jg}",
                              tag=f"kT{dp}_{jg}") for jg in range(4)]
                     for dp in range(DPT)]
            # v packed per j-group of 4: v4_sb[jg][p, jj*DL + d'] holds
            # v[jg*512 + jj*128 + p, d']
            v4_sb = [sb.tile([P, 4 * DL], bf16, name=f"v{jg}", tag=f"v{jg}")
                     for jg in range(4)]
            # xT packed head-major: xT_sb[hp] partitions = [h=2hp d 0..63,
            # h=2hp+1 d 0..63], matching the wo row layout.
            xT_sb = [sb.tile([P, S], bf16, name=f"xT{hp}", tag=f"xT{hp}")
                     for hp in range(DPT)]
            # vT staging is per (i4, dp): a [128, 512] tile lives only from
            # the bias-add to its DMA-transpose.
            vt_cur = {}

            # ---------------- building blocks ----------------
            def load_in_chunk(win, nm, i4):
                # two DMAs (4 m-blocks each): finer DMA->matmul pipelining
                t = inp.tile([P, MT * 512], bf16, name=f"{nm}in{i4}",
                             tag="pin")
                src = win.ap().rearrange("(t p) c -> p t c", p=P)
                for half in range(2):
                    nc.sync.dma_start(
                        t[:, half * 4 * 512:(half + 1) * 4 * 512]
                        .rearrange("p (t c) -> p t c", t=4),
                        src[:, half * 4:(half + 1) * 4,
                            i4 * 512:(i4 + 1) * 512])
                return t

            def proj_chunk(nm, ch, i4, w_sb, b_sb):
                # q/k projection psums share the "ps" tag (bufs=2) so chunk
                # n+1's matmuls overlap chunk n's bias-adds; v psums use the
                # "px" slot so they don't sit between the early exps and the
                # pair-0 ih=1 scores in the "ps" slot cycle.
                pool, tag = (ppx, "px") if nm == "v" else (pps, "ps")
                ps = pool.tile([P, 1024], f32, name=f"ps{nm}{i4}", tag=tag)
                for dp in range(DPT):
                    for m in range(MT):
                        nc.tensor.matmul(
                            ps[:, dp * 512:(dp + 1) * 512],
                            w_slice(w_sb, m, dp),
                            ch[:, m * 512:(m + 1) * 512],
                            start=(m == 0), stop=(m == MT - 1))
                for dp in range(DPT):
                    if nm == "q":
                        dst = qT_sb[dp][i4 // 2][:, (i4 % 2) * 512:
                                                 (i4 % 2) * 512 + 512]
                    elif nm == "k":
                        dst = kT_sb[dp][i4][:]
                    else:
                        t = vtp.tile([P, 512], bf16, name=f"vT{i4}_{dp}",
                                     tag="vt", bufs=4)
                        vt_cur[(i4, dp)] = t
                        dst = t[:]
                    nc.vector.tensor_scalar_add(
                        dst, ps[:, dp * 512:(dp + 1) * 512],
                        b_sb[:, dp:dp + 1])

            def scores_exp(h, j, ih):
                hp, hh = divmod(h, 2)
                base = hh * 64
                jg, jr = divmod(j, 4)
                a = attp.tile([P, 1024], bf16, name=f"att{h}_{j}_{ih}",
                              tag="att")
                ps = pps.tile([P, 1024], f32, name=f"pss{h}_{j}_{ih}",
                              tag="ps")
                for i5 in range(2):
                    io = i5 * 512
                    nc.tensor.matmul(
                        ps[:, io:io + 512],
                        kT_sb[hp][jg][base:base + 64, jr * P:(jr + 1) * P],
                        qT_sb[hp][ih][base:base + 64, io:io + 512],
                        start=True, stop=True)
                rsh = stat.tile([P, 1], f32, name=f"rsh{h}_{j}_{ih}",
                                tag="rsh", bufs=100)
                nc.scalar.activation(a[:], ps[:], AF.Exp, scale=SCALE,
                                     accum_out=rsh[:])
                return a, rsh

            def scores_exp_half(h, j, ih, i5, a):
                # 512-wide variant for the warm-up prefetch: lets the first
                # exps start after only q i4=0 and k i4=0 have projected.
                hp, hh = divmod(h, 2)
                base = hh * 64
                jg, jr = divmod(j, 4)
                io = i5 * 512
                ps = pps.tile([P, 512], f32, name=f"psh{h}_{j}_{ih}_{i5}",
                              tag="ps")
                nc.tensor.matmul(
                    ps[:],
                    kT_sb[hp][jg][base:base + 64, jr * P:(jr + 1) * P],
                    qT_sb[hp][ih][base:base + 64, io:io + 512],
                    start=True, stop=True)
                rsh = stat.tile([P, 1], f32, name=f"rshh{h}_{j}_{ih}_{i5}",
                                tag="rsh", bufs=100)
                nc.scalar.activation(a[:, io:io + 512], ps[:], AF.Exp,
                                     scale=SCALE, accum_out=rsh[:])
                return rsh

            def finish_strip(h, j, xps, ah, rs_halves):
                # xps: pair PSUM tile [128, S]; even head -> partitions 0:64,
                # odd head -> 64:128 (tile_position picked up automatically
                # from out.base_partition()).
                jg, jr = divmod(j, 4)
                po = (h % 2) * 64
                acc = rs_halves[0]
                for n, part in enumerate(rs_halves[1:]):
                    rs = stat.tile([P, 1], f32, name=f"rs{h}_{j}_{n}",
                                   tag="rs")
                    nc.vector.tensor_add(rs[:], acc[:], part[:])
                    acc = rs
                rs = acc
                rc = stat.tile([P, 1], f32, name=f"rc{h}_{j}", tag="rc")
                nc.vector.reciprocal(rc[:], rs[:])
                vsc = attp.tile([P, 64], bf16, name=f"vsc{h}_{j}", tag="vsc",
                                bufs=4)
                nc.vector.tensor_scalar_mul(
                    vsc[:],
                    v4_sb[jg][:, jr * DL + h * 64:jr * DL + (h + 1) * 64],
                    rc[:])
                for i5 in range(4):
                    io = (i5 % 2) * 512
                    nc.tensor.matmul(
                        xps[po:po + 64, i5 * 512:(i5 + 1) * 512], vsc[:],
                        ah[i5 // 2][:, io:io + 512],
                        start=(j == 0), stop=(j == SJ - 1),
                        skip_group_check=True)

            # ---------------- phase 1: q/k projections + ih=0 scores -------
            # Emission order is the PSUM slot order, so the early exps are
            # emitted between projection chunks as 512-wide halves, each as
            # soon as the qT/kT pieces it needs have projected.
            load_w_half(wq_sb, wq_d, DL, 0)
            load_w_half(wq_sb, wq_d, DL, 1)
            ch_q0 = load_in_chunk(qT_in, "q", 0)
            load_w_half(wk_sb, wk_d, DL, 0)
            load_w_half(wk_sb, wk_d, DL, 1)
            ch_k0 = load_in_chunk(kT_in, "k", 0)
            ch_q1 = load_in_chunk(qT_in, "q", 1)
            load_biases()

            saved = {}

            def emit_ih0_half(jg, i5):
                for j in range(jg * 4, jg * 4 + 4):
                    for h in (0, 1):
                        if (h, j) not in saved:
                            a = attp.tile([P, 1024], bf16,
                                          name=f"att{h}_{j}_0", tag="att")
                            saved[(h, j)] = (a, [])
                        a, parts = saved[(h, j)]
                        parts.append(scores_exp_half(h, j, 0, i5, a))

            def emit_ih0(js):
                for j in js:
                    for h in (0, 1):
                        a, rsh = scores_exp(h, j, 0)
                        saved[(h, j)] = (a, [rsh])

            proj_chunk("q", ch_q0, 0, wq_sb, bq_sb)
            proj_chunk("k", ch_k0, 0, wk_sb, bk_sb)
            emit_ih0_half(0, 0)
            proj_chunk("q", ch_q1, 1, wq_sb, bq_sb)
            emit_ih0_half(0, 1)
            # remaining k chunks come before q i4=2,3: every ih=0 exp only
            # needs kT plus the first qT half, while the q tail is not needed
            # until the pair-0 main loop.
            # saved2: a few of pair 1's ih=0 exps pulled into phase-1 ACT
            # idle windows (they only need kT groups already projected).
            saved2 = {}

            def emit_pair1_ih0(h, j):
                saved2[(h, j)] = scores_exp(h, j, 0)

            ch = load_in_chunk(kT_in, "k", 1)
            proj_chunk("k", ch, 1, wk_sb, bk_sb)
            emit_ih0([4, 5, 6])
            ch = load_in_chunk(kT_in, "k", 2)
            proj_chunk("k", ch, 2, wk_sb, bk_sb)
            emit_ih0([7, 8, 9, 10])
            ch = load_in_chunk(qT_in, "q", 2)
            proj_chunk("q", ch, 2, wq_sb, bq_sb)
            emit_ih0([11])
            ch = load_in_chunk(kT_in, "k", 3)
            proj_chunk("k", ch, 3, wk_sb, bk_sb)
            emit_ih0([12, 13, 14])
            # first ih=1 half-exps (pair 0, j<4) fill the ACT window while
            # q i4=3 is still projecting
            saved1 = {}
            for j in range(4):
                for h in (0, 1):
                    a = attp.tile([P, 1024], bf16, name=f"att{h}_{j}_1",
                                  tag="att")
                    saved1[(h, j)] = (a, [scores_exp_half(h, j, 1, 0, a)])
            ch = load_in_chunk(qT_in, "q", 3)
            proj_chunk("q", ch, 3, wq_sb, bq_sb)
            emit_ih0([15])
            for j in range(4):
                for h in (0, 1):
                    a, parts = saved1[(h, j)]
                    parts.append(scores_exp_half(h, j, 1, 1, a))

            # ---------------- phase 2: v projection + transposes ----------
            load_w_half(wv_sb, wv_d, DL, 0)
            load_w_half(wv_sb, wv_d, DL, 1)
            v_chunks = [load_in_chunk(vT_in, "v", i4) for i4 in range(3)]
            # wo/bo are first needed by the pair-0 partial output projection
            # (mid-kernel); loading them here keeps the front DMA bandwidth
            # for the projection inputs.
            wo_sb = const.tile([P, DPT * DM], bf16, name="wo", tag="wo")
            nc.sync.dma_start(
                wo_sb[:].rearrange("p (t c) -> p t c", t=DPT),
                wo_d.ap().rearrange("(t p) c -> p t c", p=P))
            bo_sb = const.tile([P, DM], f32, name="bo", tag="bo")
            nc.sync.dma_start(bo_sb[:], bo_d.ap())

            for i4 in range(4):
                if i4 == 1:
                    v_chunks.append(load_in_chunk(vT_in, "v", 3))
                proj_chunk("v", v_chunks[i4], i4, wv_sb, bv_sb)
                for dp in range(DPT):
                    out_view = v4_sb[i4][:].rearrange(
                        "p (j c) -> p j c", j=4)[:, :,
                                                 dp * P:(dp + 1) * P]
                    nc.sync.dma_start(
                        out_view, vt_cur.pop((i4, dp))[:], transpose=True)

            # ---------------- phase 3: attention (head pairs) -------------
            # finish_strip (softmax divisor + AV) runs one (h, j) step behind
            # scores/exp; pair 0's partial output projection (bf16, +bo) is
            # interleaved into pair 1's loop and streamed to DRAM there --
            # the tail only has pair 1's partial left to write. The host
            # sums the two bf16 partials in fp32.
            def o0_group(jt):
                ot = outp.tile([P, DM], bf16, name=f"o0_{jt}", tag="ot")
                ps = pps.tile([P, DM], f32, name=f"pso0_{jt}", tag="ps")
                for n5 in range(2):
                    no = n5 * 512
                    nc.tensor.matmul(
                        ps[:, no:no + 512], xT_sb[0][:, jt * P:(jt + 1) * P],
                        wo_sb[:, no:no + 512], start=True, stop=True)
                nc.vector.tensor_add(ot[:], ps[:], bo_sb[:])
                eng = (nc.sync, nc.scalar)[jt % 2]
                eng.dma_start(O0_d.ap()[jt * P:(jt + 1) * P, :], ot[:])

            for hp in range(DPT):
                h0, h1 = 2 * hp, 2 * hp + 1
                xps = ppx.tile([P, S], f32, name=f"xps{hp}", tag="px")
                pend = []
                step = 0
                for j in range(SJ):
                    for h in (h0, h1):
                        if hp == 0:
                            a0, r0parts = saved.pop((h, j))
                        elif (h, j) in saved2:
                            a0, rsh0 = saved2.pop((h, j))
                            r0parts = [rsh0]
                        else:
                            a0, rsh0 = scores_exp(h, j, 0)
                            r0parts = [rsh0]
                        if hp == 0 and (h, j) in saved1:
                            a1, r1parts = saved1.pop((h, j))
                        else:
                            a1, rsh1 = scores_exp(h, j, 1)
                            r1parts = [rsh1]
                        pend.append((h, j, a0, a1, r0parts + r1parts))
                        if len(pend) > 1:
                            ph, pj, b0, b1, rparts = pend.pop(0)
                            finish_strip(ph, pj, xps, [b0, b1], rparts)
                        if hp == 1 and step % 2 == 0 and step // 2 < SJ:
                            o0_group(step // 2)
                        step += 1
                for ph, pj, b0, b1, rparts in pend:
                    finish_strip(ph, pj, xps, [b0, b1], rparts)
                for c4 in range(4):
                    dst = xT_sb[hp][:, c4 * 512:(c4 + 1) * 512]
                    src = xps[:, c4 * 512:(c4 + 1) * 512]
                    if hp == 1 and c4 % 2 == 1:
                        # ACT is idle once the last exp is done
                        nc.scalar.activation(dst, src, AF.Copy)
                    else:
                        nc.vector.tensor_copy(dst, src)

            # ---------------- phase 4: output tail ------------------------
            # Pair 1's partial only: copies split DVE/ACT, bf16 DMAs rotated
            # across queues.
            for jt in range(SJ):
                ot = outp.tile([P, DM], bf16, name=f"ot{jt}", tag="ot")
                ps = pps.tile([P, DM], f32, name=f"pso1_{jt}", tag="ps")
                for n5 in range(2):
                    no = n5 * 512
                    nc.tensor.matmul(
                        ps[:, no:no + 512],
                        xT_sb[1][:, jt * P:(jt + 1) * P],
                        wo_sb[:, DM + no:DM + no + 512],
                        start=True, stop=True)
                if jt % 2 == 0:
                    nc.vector.tensor_copy(ot[:], ps[:])
                else:
                    nc.scalar.activation(ot[:], ps[:], AF.Copy)
                eng = (nc.sync, nc.gpsimd)[jt % 2]
                eng.dma_start(O1_d.ap()[jt * P:(jt + 1) * P, :], ot[:])

    if split_waits:
        _split_excess_waits(nc)
    return nc


def _get_program():
    global _PROGRAM
    if _PROGRAM is None:
        _PROGRAM = build_program()
    return _PROGRAM


def shard_inputs(inputs):
    """FULL inputs -> per-core in_maps (list of 8 dicts)."""
    import ml_dtypes

    def _bf16(x):
        return np.ascontiguousarray(np.asarray(x, np.float32)).astype(
            ml_dtypes.bfloat16)

    q = np.asarray(inputs["query"], dtype=np.float32)
    k = np.asarray(inputs["key"], dtype=np.float32)
    v = np.asarray(inputs["value"], dtype=np.float32)
    Wq = np.asarray(inputs["Wq"], dtype=np.float32)
    Wk = np.asarray(inputs["Wk"], dtype=np.float32)
    Wv = np.asarray(inputs["Wv"], dtype=np.float32)
    Wo = np.asarray(inputs["Wo"], dtype=np.float32)
    bq = np.asarray(inputs["bq"], dtype=np.float32)
    bk = np.asarray(inputs["bk"], dtype=np.float32)
    bv = np.asarray(inputs["bv"], dtype=np.float32)
    bo = np.asarray(inputs["bo"], dtype=np.float32)

    qT = [_bf16(q[b].T) for b in range(B)]
    kT = [_bf16(k[b].T) for b in range(B)]
    vT = [_bf16(v[b].T) for b in range(B)]

    in_maps = []
    for c in range(N_CORES):
        b, g = c // GROUPS, c % GROUPS
        sl = slice(g * DL, (g + 1) * DL)
        bo_bc = (np.ascontiguousarray(np.broadcast_to(bo, (P, DM)))
                 if g == 0 else np.zeros((P, DM), np.float32))
        in_maps.append({
            "qT_in": qT[b],
            "kT_in": kT[b],
            "vT_in": vT[b],
            "wq": _bf16(Wq[:, sl]),
            "wk": _bf16(Wk[:, sl]),
            "wv": _bf16(Wv[:, sl]),
            "wo": _bf16(Wo[sl, :]),
            "bq": np.ascontiguousarray(bq[sl].reshape(DL, 1)),
            "bk": np.ascontiguousarray(bk[sl].reshape(DL, 1)),
            "bv": np.ascontiguousarray(bv[sl].reshape(DL, 1)),
            "bo_bc": bo_bc,
        })
    return in_maps


def unshard_output(results):
    """results: list of 8 dicts with bf16 partials 'O0'/'O1' [S, DM] ->
    full [B, S, DM] fp32."""
    out = np.zeros((B, S, DM), np.float32)
    for c in range(N_CORES):
        b = c // GROUPS
        out[b] += np.asarray(results[c]["O0"], np.float32)
        out[b] += np.asarray(results[c]["O1"], np.float32)
    return out


def kernel(**inputs):
    nc = _get_program()
    in_maps = shard_inputs(inputs)
    res = run_bass_kernel_spmd(nc, in_maps, core_ids=list(range(N_CORES)))
    return unshard_output(res.results)
